# revision 34
# baseline (speedup 1.0000x reference)
"""Trainium2 Bass kernel for AttentionAggregate (GAT-style neighbor aggregation).

Reference computation (per node n, neighbors k=0..K-1):
    pt = target @ W.T + b                      # [N, D]
    pm = middle @ W.T + b                      # [N, K, D]
    score = leaky_relu((pt[:,None,:] + pm) @ a_w.T + a_b)
    coef  = softmax(score, axis=K)
    out   = sum_k coef * middle                # [N, D]

Key algebraic simplification: the W-projection only enters through the dot
with a_w, so with u = a_w @ W (a single D-vector) and c = 2*(a_w.b) + a_b:
    score[n,k] = target[n].u + middle[n,k].u + c
This removes all large matmuls; the kernel is a memory-bound pass over
`middle` (512 MiB) with per-node softmax weighting.

Sharding: data-parallel over nodes. N=16384 nodes split across 8 cores
(2048 nodes each); W/b/a_w/a_b replicated; no cross-core communication.

Engine assignment per 128-node tile (node on partition), sized so every
engine fits under the ~12.7 us/tile DMA slot (4 MiB HBM read at ~330 GB/s):
  Pool: issues the middle-tile DMAs as casting SWDGE transfers — DMA
        converts f32 (HBM) -> fp16 (SBUF) in flight, so no on-chip convert
        pass is needed. fp16 (10 mantissa bits) keeps rounding ~4x below
        bf16; all values here are O(+-10), well inside fp16 range.
  DVE:  one big fp16 multiply m2 = mid*u (2x perf mode for 16-bit) + one
        segmented fp16 reduce -> s[n,k], plus bias/leaky/reciprocal smalls.
  ACT:  exp with fused denominator accumulation, 32x diag(e_k) generation
        (Copy activation with per-partition scale), PSUM evacuation scaled
        by 1/den.
  PE:   32x accumulating fp16 matmuls diag(e_k) @ mid[:,k,:] (1 cycle/row).
  Sync: setup + output DMAs on the SP HWDGE queue.

Softmax is computed without max-subtraction: scores are O(+-8) here, exp
stays well inside f32 range.
"""

from contextlib import ExitStack

import numpy as np

import concourse.bass as bass
import concourse.tile as tile
from concourse import mybir
from concourse.bass_utils import run_bass_kernel_spmd

N_CORES = 8
N, K, D = 16384, 32, 256
NS = N // N_CORES  # nodes per core
P = 128
F32 = mybir.dt.float32
F16 = mybir.dt.float16
ALU = mybir.AluOpType
AF = mybir.ActivationFunctionType
AX = mybir.AxisListType
NEG_SLOPE = 0.01


def emit_kernel(tc, out, tgt, mid, W, b, a_w, a_b, ident, ns):
    nc = tc.nc
    nt = ns // P  # node tiles per core
    with ExitStack() as ctx:
        singles = ctx.enter_context(tc.tile_pool(name="singles", bufs=1))
        mids = ctx.enter_context(tc.tile_pool(name="mids", bufs=5))
        small = ctx.enter_context(tc.tile_pool(name="small", bufs=6))
        scr = ctx.enter_context(tc.tile_pool(name="scr", bufs=1))
        dgs = ctx.enter_context(tc.tile_pool(name="dgs", bufs=4))
        psum = ctx.enter_context(tc.tile_pool(name="psum", bufs=2, space="PSUM"))
        opsum = ctx.enter_context(tc.tile_pool(name="opsum", bufs=2, space="PSUM"))
        outs = ctx.enter_context(tc.tile_pool(name="outs", bufs=3))

        # Tile 0's cast-DMA goes FIRST so its ~11us transfer overlaps all of
        # the setup chain; the small setup DMAs follow right behind it.
        m0 = mids.tile([P, K, D], F16, tag="mid")
        nc.gpsimd.dma_start(m0, mid[0:P, :, :])

        # ---- setup: u = a_w @ W, c = 2*(a_w.b) + a_b ----
        # Setup DMAs ride the Pool SWDGE queue (on the sync HWDGE queue they
        # get starved for ~30us by the SWDGE cast stream).
        W0 = singles.tile([P, D], F32)
        W1 = singles.tile([P, D], F32)
        nc.gpsimd.dma_start(W0, W[0:P, :])
        nc.gpsimd.dma_start(W1, W[P : 2 * P, :])
        # a_w transposed onto partitions: awT[p, g] = a_w[0, g*128 + p]
        awT = singles.tile([P, 2], F32)
        nc.gpsimd.dma_start(awT, a_w.rearrange("o (g p) -> p (g o)", g=2))
        b_row = singles.tile([1, D], F32)
        nc.gpsimd.dma_start(b_row, b.unsqueeze(0))
        aw_row = singles.tile([1, D], F32)
        nc.gpsimd.dma_start(aw_row, a_w)
        ab_t = singles.tile([1, 1], F32)
        nc.gpsimd.dma_start(ab_t, a_b.unsqueeze(0))
        id_h = singles.tile([P, P], F16)
        nc.gpsimd.dma_start(id_h, ident)
        # target, all tiles at once: tg_all[p, t, d] = tgt[t*128+p, d]
        # (cast to fp16 in the DMA; feeds a fp16 2x dot against u)
        tg_all = singles.tile([P, nt, D], F16)
        nc.gpsimd.dma_start(tg_all, tgt.rearrange("(t p) d -> p t d", p=P))

        # Wsc[d, e] = a_w[d] * W[d, e]
        Wsc0 = singles.tile([P, D], F32)
        Wsc1 = singles.tile([P, D], F32)
        nc.vector.tensor_scalar_mul(Wsc0, W0, awT[:, 0:1])
        nc.vector.tensor_scalar_mul(Wsc1, W1, awT[:, 1:2])
        ones_col = singles.tile([P, 1], F32)
        ones_row = singles.tile([1, P], F32)
        nc.vector.memset(ones_col, 1.0)
        nc.vector.memset(ones_row, 1.0)
        # u[e] = sum_d Wsc[d, e]  (partition reduction via PE)
        u_ps = psum.tile([1, D], F32)
        nc.tensor.matmul(u_ps, ones_col, Wsc0, start=True, stop=False)
        nc.tensor.matmul(u_ps, ones_col, Wsc1, start=False, stop=True)
        u_row = singles.tile([1, D], F32)
        nc.scalar.copy(u_row, u_ps)

        # c = 2*(b . a_w) + a_b   (fused mul+reduce)
        baw_scr = scr.tile([1, D], F32, tag="baw_scr")
        baw = singles.tile([1, 1], F32)
        nc.vector.scalar_tensor_tensor(
            out=baw_scr, in0=b_row, scalar=0.0, in1=aw_row,
            op0=ALU.bypass, op1=ALU.mult, accum_out=baw,
        )
        c_s = singles.tile([1, 1], F32)
        nc.scalar.activation(c_s, baw, AF.Identity, bias=ab_t, scale=2.0)

        # broadcast u, c across all 128 partitions via PE outer product
        ub_ps = psum.tile([P, D], F32)
        nc.tensor.matmul(ub_ps, ones_row, u_row, start=True, stop=True)
        u_h = singles.tile([P, D], F16)
        nc.scalar.copy(u_h, ub_ps)
        cb_ps = psum.tile([P, 1], F32)
        nc.tensor.matmul(cb_ps, ones_row, c_s, start=True, stop=True)
        c_b = singles.tile([P, 1], F32)
        nc.scalar.copy(c_b, cb_ps)

        # per-node constants: stc_c[:, t] = target[t].u + c and 0.01x it.
        # The leaky-relu folds into two ACT exps via
        #   exp(leaky(x)) = max(exp(x), exp(0.01x));
        # with x = s + stc_c, the second exp is exp(0.01*s + 0.01*stc_c).
        stc = singles.tile([P, nt], F16)
        tg_scr = scr.tile([P, nt, D], F16, tag="tg_scr")
        tg_tr = scr.tile([P, nt, D // 2], F16, tag="tg_tr")
        nc.vector.tensor_mul(
            tg_scr, tg_all, u_h.unsqueeze(1).broadcast_to([P, nt, D])
        )
        nc.vector.tensor_add(
            tg_tr, tg_scr[:, :, 0 : D // 2], tg_scr[:, :, D // 2 : D]
        )
        with nc.allow_low_precision("fp16 scores, tolerance is 2e-2"):
            nc.vector.reduce_sum(stc, tg_tr, AX.X)
        stc_c = singles.tile([P, nt], F32)
        nc.vector.tensor_scalar_add(stc_c, stc, c_b)
        stc_c001 = singles.tile([P, nt], F32)
        nc.vector.tensor_scalar_mul(stc_c001, stc_c, 0.01)

        m2h_scr = scr.tile([P, K, D], F16, tag="m2h_scr")
        tr_a = scr.tile([P, K, D // 2], F16, tag="tr_a")
        tr_b = scr.tile([P, K, D // 4], F16, tag="tr_b")
        tr_c = scr.tile([P, K, D // 8], F16, tag="tr_c")
        tr_d = scr.tile([P, K, D // 16], F16, tag="tr_d")
        u_h_bc = u_h.unsqueeze(1).broadcast_to([P, K, D])

        # identity replicated K times along free: idK[p, k, q] = (p == q)
        idK = singles.tile([P, K, P], F16)
        nc.vector.tensor_copy(idK, id_h.unsqueeze(1).broadcast_to([P, K, P]))

        def phase1(t):
            """Stream in tile t (cast DMA) and compute raw scores s[:, k]."""
            if t == 0:
                m = m0
            else:
                m = mids.tile([P, K, D], F16, tag="mid")
                nc.gpsimd.dma_start(m, mid[t * P : (t + 1) * P, :, :])
            s = small.tile([P, K], F16, tag="s")
            nc.vector.tensor_mul(m2h_scr, m, u_h_bc)
            h = D // 2
            nc.vector.tensor_add(tr_a, m2h_scr[:, :, 0:h], m2h_scr[:, :, h:D])
            nc.vector.tensor_add(
                tr_b, tr_a[:, :, 0 : h // 2], tr_a[:, :, h // 2 : h]
            )
            nc.vector.tensor_add(
                tr_c, tr_b[:, :, 0 : h // 4], tr_b[:, :, h // 4 : h // 2]
            )
            nc.vector.tensor_add(
                tr_d, tr_c[:, :, 0 : h // 8], tr_c[:, :, h // 8 : h // 4]
            )
            with nc.allow_low_precision("fp16 scores, tolerance is 2e-2"):
                nc.vector.reduce_sum(s, tr_d, AX.X)
            return m, s

        def finish(t, m, s):
            """Softmax (exp-max leaky), diag build, PE aggregation, out."""
            # e = exp(leaky(s + stc_c)) = max(exp(s + A), exp(0.01(s + A)))
            e1 = small.tile([P, K], F16, tag="e1")
            e2x = small.tile([P, K], F16, tag="e2x")
            nc.scalar.activation(
                e1, s, AF.Exp, bias=stc_c[:, t : t + 1], scale=1.0
            )
            nc.scalar.activation(
                e2x, s, AF.Exp, bias=stc_c001[:, t : t + 1], scale=0.01
            )
            e = small.tile([P, K], F32, tag="e")
            nc.vector.tensor_max(e, e1, e2x)
            # pack e into pairs on ACT (denominator fused into the first
            # copy); e2's packed innermost dim lets the dgall op hit 2x.
            e2 = small.tile([P, K, 2], F16, tag="e2")
            den = small.tile([P, 1], F32, tag="den")
            nc.scalar.activation(
                e2[:, :, 0:1], e.unsqueeze(2), AF.Copy, accum_out=den
            )
            nc.scalar.copy(e2[:, :, 1:2], e.unsqueeze(2))
            rcp = small.tile([P, 1], F32, tag="rcp")
            nc.vector.reciprocal(rcp, den)

            # all 32 diag blocks in ONE DVE op (2x): idK * e-broadcast
            dgall = dgs.tile([P, K, P], F16, tag="dgall")
            nc.vector.tensor_mul(
                dgall.rearrange("p k (j i) -> p k j i", i=2),
                idK.rearrange("p k (j i) -> p k j i", i=2),
                e2.unsqueeze(2).broadcast_to([P, K, P // 2, 2]),
            )
            o_ps = opsum.tile([P, D], F32, tag="o_ps")
            for k in range(K):
                nc.tensor.matmul(
                    o_ps, dgall[:, k, :], m[:, k, :],
                    start=(k == 0), stop=(k == K - 1), skip_group_check=True,
                )
            o_sb = outs.tile([P, D], F32, tag="o_sb")
            nc.scalar.mul(o_sb, o_ps, rcp[:, 0:1])
            nc.sync.dma_start(out[t * P : (t + 1) * P, :], o_sb)

        # ---- main loop, software-pipelined by one stage: tile t's softmax/
        # aggregation is emitted after tile t+1's phase-1 so the ACT round
        # trip (exp) never bubbles the DVE.
        prev = None
        for t in range(nt):
            cur = (t, *phase1(t))
            if prev is not None:
                finish(*prev)
            prev = cur
        finish(*prev)


def build_nc(ns=NS):
    nc = bass.Bass("TRN2", debug=False, num_devices=N_CORES)
    tgt = nc.dram_tensor("target", [ns, D], F32, kind="ExternalInput").ap()
    mid = nc.dram_tensor("middle", [ns, K, D], F32, kind="ExternalInput").ap()
    W = nc.dram_tensor("W", [D, D], F32, kind="ExternalInput").ap()
    b = nc.dram_tensor("b", [D], F32, kind="ExternalInput").ap()
    a_w = nc.dram_tensor("a_w", [1, D], F32, kind="ExternalInput").ap()
    a_b = nc.dram_tensor("a_b", [1], F32, kind="ExternalInput").ap()
    ident = nc.dram_tensor("ident", [P, P], F32, kind="ExternalInput").ap()
    out = nc.dram_tensor("out", [ns, D], F32, kind="ExternalOutput").ap()
    with tile.TileContext(nc) as tc:
        emit_kernel(tc, out, tgt, mid, W, b, a_w, a_b, ident, ns)
    import bass_rust as _br

    # Split multi-wait instructions (walrus allows at most 1 sync wait per
    # instruction; Tile can emit more after multi-DMA dependencies).
    _br.generate_event_semaphores(nc)
    return nc


_NC_CACHE = {}


def _get_nc(ns=NS):
    if ns not in _NC_CACHE:
        _NC_CACHE[ns] = build_nc(ns)
    return _NC_CACHE[ns]


def make_in_maps(target, middle, W, b, a_w, a_b):
    target = np.ascontiguousarray(np.asarray(target, dtype=np.float32))
    middle = np.ascontiguousarray(np.asarray(middle, dtype=np.float32))
    W = np.ascontiguousarray(np.asarray(W, dtype=np.float32))
    b = np.ascontiguousarray(np.asarray(b, dtype=np.float32))
    a_w = np.ascontiguousarray(np.asarray(a_w, dtype=np.float32))
    a_b = np.ascontiguousarray(np.asarray(a_b, dtype=np.float32))
    ident = np.eye(P, dtype=np.float32)
    tgt_shards = np.split(target, N_CORES, axis=0)
    mid_shards = np.split(middle, N_CORES, axis=0)
    return [
        {
            "target": tgt_shards[i],
            "middle": mid_shards[i],
            "W": W,
            "b": b,
            "a_w": a_w,
            "a_b": a_b,
            "ident": ident,
        }
        for i in range(N_CORES)
    ]


def run_sharded(in_maps, **kwargs):
    nc = _get_nc(in_maps[0]["target"].shape[0])
    res = run_bass_kernel_spmd(nc, in_maps, list(range(N_CORES)), **kwargs)
    full = np.concatenate([r["out"] for r in res.results], axis=0)
    return full, res


def kernel(target, middle, W, b, a_w, a_b):
    in_maps = make_in_maps(target, middle, W, b, a_w, a_b)
    full, _ = run_sharded(in_maps)
    return full
